# revision 18
# baseline (speedup 1.0000x reference)
"""Multi-head causal attention (B=2,S=2048,D=768,H=12) on 8 NeuronCores.

Sharding: core = (batch, head_group) with 2 batches x 4 head groups of 3
heads.  Each core computes q/k/v projections for its heads, causal
attention, and a partial output projection (wo rows for its heads); the
host sums the 4 partials per batch and adds bo (+ folded bv @ wo).

Fast path (causal mask, zero q/k biases):
  - x is pre-transposed on the host; all loads are plain contiguous DMAs
    issued weights-first so the first matmul starts ~4us in.
  - Q/K projections packed into 3 M-tiles: [h0q|h1q], [h0k|h1k],
    [h2q|h2k]; a partition-swapped copy of the last enables row-paired
    h2 QK^T matmuls.
  - QK^T row-paired (two 64-contraction matmuls in disjoint PE row
    groups run concurrently); diagonal 128-blocks are column-trimmed in
    QK^T, exp, and PV.
  - softmax denominators: ones-column in V (free rowsum), fast DVE
    reciprocal, fp16 PE broadcast, one DVE multiply per head.
  - causal tril masking on GpSimd; PSUM time-shared 4/2/2 banks.
"""

import numpy as np

import bass_rust
import concourse.bass as bass
import concourse.mybir as mybir
import concourse.tile as tile
from concourse.bass_utils import run_bass_kernel_spmd

F16 = mybir.dt.float16
F32 = mybir.dt.float32

B, S, D = 2, 2048, 768
H, DK = 12, 64
HPC = 3            # heads per core
N_CORES = 8
QB = 512           # query block (psum free dim)
NQB = S // QB      # 4
NKT = S // 128     # 16 key tiles
DKT = D // 128     # 6 contraction tiles for projections

ScopedClock = bass_rust.ScopedClock


# ---------------------------------------------------------------------------
# walrus in this build accepts at most ONE sync-wait per instruction; spread
# extra waits onto NOPs placed immediately before the owning instruction.

def _split_drain_and_barrier(self, tick_clock, wait_clock):
    probe = self.nc.sync.nop()
    wait_clock.add_sem_waits(probe.ins, ScopedClock({None: tick_clock.global_clock}))
    si = probe.ins.sync_info
    waits = list(si.on_wait) if si is not None else []
    if len(waits) > 1:
        si.on_wait = waits[:1]
        for w in waits[1:]:
            n = self.nc.sync.nop()
            nsi = n.ins.sync_info
            if nsi is None:
                n.ins.sync_info = bass_rust.SyncInfo(on_wait=[w], on_update=[])
            else:
                nsi.on_wait = [w]
    self.nc.sync.drain()

    self.nc.all_engine_barrier()
    assert self.sems is not None
    popped = self.nc._tile_sem_poison_stack.pop()
    assert popped is self._sem_poison
    self.nc.clear_and_free_semaphores(list(self.sems.allocated().values()))
    self.nc.all_engine_barrier()


tile.TileContext._drain_and_barrier = _split_drain_and_barrier

_nop_ctr = [0]


def split_multi_waits(nc):
    def visit(parent):
        for bb in parent.blocks:
            insts = bb.instructions
            out = []
            changed = False
            for inst in insts:
                si = inst.sync_info
                if si is not None and len(si.on_wait) > 1:
                    waits = list(si.on_wait)
                    for w in waits[:-1]:
                        _nop_ctr[0] += 1
                        nop = mybir.InstNoOp(
                            name=f"wsplit{_nop_ctr[0]}",
                            sync_info=mybir.SyncInfo(on_wait=[w], on_update=[]),
                            bass_nofuse=True,
                            engine=inst.engine,
                        )
                        out.append(nop)
                    si.on_wait = waits[-1:]
                    changed = True
                out.append(inst)
            if changed:
                bb.instructions = out
    for f in nc.m.functions:
        visit(f)


# ---------------------------------------------------------------------------
# fast causal kernel


def build_nc_fast(with_qk_bias: bool = False):
    nc = bass.Bass("TRN2", target_bir_lowering=False, debug=False,
                   num_devices=N_CORES)

    xTd = nc.dram_tensor("xT", (128, DKT, S), F16, kind="ExternalInput").ap()
    wqkd = nc.dram_tensor("wqk", (128, DKT, 3, 128), F16, kind="ExternalInput").ap()
    wvd = nc.dram_tensor("wv3", (128, DKT, HPC * DK), F16, kind="ExternalInput").ap()
    wo2d = nc.dram_tensor("wo2", (128, D), F16, kind="ExternalInput").ap()
    wo3d = nc.dram_tensor("wo3", (DK, D), F16, kind="ExternalInput").ap()
    if with_qk_bias:
        bqkd = nc.dram_tensor("bqk3", (128, 3), F32, kind="ExternalInput").ap()
    o16 = nc.dram_tensor("o16", (S, D), F16, kind="ExternalOutput").ap()

    with tile.TileContext(nc) as tc:
        cst = tc.alloc_tile_pool(name="cst", bufs=1)
        psA2 = tc.alloc_tile_pool(name="psA2", bufs=2, space="PSUM")  # 4 banks
        psA = tc.alloc_tile_pool(name="psA", bufs=2, space="PSUM")    # 2 banks
        psC = tc.alloc_tile_pool(name="psC", bufs=2, space="PSUM")    # 2 banks
        c1s_pool = tc.alloc_tile_pool(name="c1s", bufs=2)
        osb_pool = tc.alloc_tile_pool(name="osbp", bufs=3)

        # ---- constants / weights first (small), then x chunks
        wqk_sb = cst.tile([128, DKT, 3, 128], F16, tag="wqk")
        wv_sb = cst.tile([128, DKT, HPC * DK], F16, tag="wv")
        wo2_sb = cst.tile([128, D], F16, tag="wo2")
        wo3_sb = cst.tile([DK, D], F16, tag="wo3")
        for m in (1, 0, 2):
            nc.sync.dma_start(wqk_sb[:, :, m, :], wqkd[:, :, m, :])
        nc.sync.dma_start(wv_sb[:], wvd)
        nc.sync.dma_start(wo2_sb[:], wo2d)
        nc.sync.dma_start(wo3_sb[:], wo3d)
        if with_qk_bias:
            bqk_sb = cst.tile([128, 3], F32, tag="bqk")
            nc.sync.dma_start(bqk_sb[:], bqkd)

        xt = cst.tile([128, DKT, S], F16, tag="xt")
        for c in range(DKT):
            eng = nc.scalar if c % 2 == 0 else nc.gpsimd
            eng.dma_start(xt[:, c, :], xTd[:, c, :])

        tril = cst.tile([128, 128], F16, tag="tril")
        nc.gpsimd.memset(tril[:], 1.0)
        # keep (f - p >= 0) i.e. q >= k, else 0
        nc.gpsimd.affine_select(
            out=tril[:], in_=tril[:], compare_op=mybir.AluOpType.is_ge,
            fill=0.0, base=0, pattern=[[1, 128]], channel_multiplier=-1)

        ones16 = cst.tile([128, DK], F16, tag="ones16")
        nc.vector.memset(ones16[:], 1.0)

        V_sb = cst.tile([128, NKT, HPC, DK + 1], F16, tag="V")
        nc.gpsimd.memset(V_sb[:, :, :, DK:DK + 1], 1.0)  # rowsum ones-column

        qT01 = cst.tile([128, S], F16, tag="qT01")
        kT01 = cst.tile([128, S], F16, tag="kT01")
        qkT2a = cst.tile([128, S], F16, tag="qkT2a")
        qkT2b = cst.tile([128, S], F16, tag="qkT2b")

        # double-buffered per i-parity so exp(i+1) overlaps PV(i)
        pts2 = [[cst.tile([128, NKT, QB], F16, tag=f"pt{h}_{par}",
                          name=f"pt{h}_{par}") for h in range(HPC)]
                for par in range(2)]

        c01 = cst.tile([128, S], F16, tag="c01")
        ctx2 = cst.tile([128, S], F16, tag="ctx2")
        rs32 = cst.tile([128, HPC, QB], F32, tag="rs32")
        rs16 = cst.tile([128, HPC, QB], F16, tag="rs16")

        # ---- emission helpers -------------------------------------------
        dsts = (qT01, kT01, qkT2a)

        def proj_chain(m, sb):
            pp = psA.tile([128, QB], F32, tag="A", name=f"pp{m}_{sb}")
            for c in range(DKT):
                nc.tensor.matmul(
                    pp[:], wqk_sb[:, c, m, :],
                    xt[:, c, sb * QB:(sb + 1) * QB],
                    start=(c == 0), stop=(c == DKT - 1))
            dcols = dsts[m][:, sb * QB:(sb + 1) * QB]
            if with_qk_bias:
                nc.vector.tensor_scalar_add(dcols, pp[:], bqk_sb[:, m:m + 1])
            else:
                nc.vector.tensor_copy(dcols, pp[:])

        def v_chain(st):
            pp = psA.tile([128, QB], F32, tag="A", name=f"ppv{st}")
            for c in range(DKT):
                nc.tensor.matmul(
                    pp[:, 0:HPC * DK], xt[:, c, st * 128:(st + 1) * 128],
                    wv_sb[:, c, :], start=(c == 0), stop=(c == DKT - 1))
            nc.vector.tensor_copy(
                V_sb[:, st, :, 0:DK],
                pp[:, 0:HPC * DK].rearrange("p (h d) -> p h d", d=DK))

        def off_of(i, j):
            return 128 * (j - 4 * i) if j >= 4 * i else 0

        def qk01(i, g2):
            q0 = i * QB
            pt0, pt1 = pts2[i % 2][0], pts2[i % 2][1]
            j0, j1 = 2 * g2, 2 * g2 + 1
            off0 = off_of(i, j0)
            sA = psA2.tile([128, 2, QB], F32, tag="A2", name=f"sA{i}_{g2}")
            sB = psA2.tile([128, 2, QB], F32, tag="A2", name=f"sB{i}_{g2}")
            for jj, j in ((0, j0), (1, j1)):
                off = off_of(i, j)
                nc.tensor.matmul(
                    sA[:, jj, off:], kT01[0:DK, j * 128:(j + 1) * 128],
                    qT01[0:DK, q0 + off:q0 + QB], start=True, stop=True)
                nc.tensor.matmul(
                    sB[:, jj, off:], kT01[DK:128, j * 128:(j + 1) * 128],
                    qT01[DK:128, q0 + off:q0 + QB], start=True, stop=True)
            # one exp per [128,2,QB] tile; cols < off1 of plane j1 are
            # garbage-but-never-read (PV/tril only touch cols >= off)
            nc.scalar.activation(pt0[:, j0:j0 + 2, off0:], sA[:, :, off0:],
                                 mybir.ActivationFunctionType.Exp)
            nc.scalar.activation(pt1[:, j0:j0 + 2, off0:], sB[:, :, off0:],
                                 mybir.ActivationFunctionType.Exp)

        def qk2(i, g2):
            q0 = i * QB
            pt2 = pts2[i % 2][2]
            j0, j1 = 2 * g2, 2 * g2 + 1
            off0, off1 = off_of(i, j0), off_of(i, j1)
            sC = psA2.tile([128, 2, QB], F32, tag="A2", name=f"sC{i}_{g2}")
            nc.tensor.matmul(
                sC[:, 0, off0:], qkT2b[0:DK, j0 * 128:(j0 + 1) * 128],
                qkT2a[0:DK, q0 + off0:q0 + QB], start=True, stop=True)
            nc.tensor.matmul(
                sC[:, 1, off1:], qkT2a[DK:128, j1 * 128:(j1 + 1) * 128],
                qkT2b[DK:128, q0 + off1:q0 + QB], start=True, stop=True)
            nc.scalar.activation(pt2[:, j0:j0 + 2, off0:], sC[:, :, off0:],
                                 mybir.ActivationFunctionType.Exp)

        pvs_of = {}

        def mask_pv(i):
            """tril mask (GpSimd) + PV accumulation chains (PE)."""
            kt = 4 * (i + 1)
            pts = pts2[i % 2]
            for h in range(HPC):
                for jj in range(4):
                    j = 4 * i + jj
                    off = 128 * jj
                    nc.gpsimd.tensor_mul(
                        pts[h][:, j, off:off + 128],
                        pts[h][:, j, off:off + 128], tril[:])
            pvs = []
            for h in range(HPC):
                pv = psC.tile([128, QB], F32, tag="C", name=f"pv{i}_{h}")
                pvs.append(pv)
                for j in range(kt):
                    off = off_of(i, j)
                    nc.tensor.matmul(
                        pv[0:DK + 1, off:QB], V_sb[:, j, h, :],
                        pts[h][:, j, off:QB],
                        start=(j == 0), stop=(j == kt - 1),
                        skip_group_check=True)
            pvs_of[i] = pvs

        def recip_bcmul(i):
            """1/r = exp(-ln r) on ACT (same table as the exp stream), PE
            broadcast, SBUF stage, DVE normalize multiply."""
            q0 = i * QB
            pvs = pvs_of[i]
            for h in range(HPC):
                nc.scalar.activation(rs32[DK:DK + 1, h, :],
                                     pvs[h][DK:DK + 1, :],
                                     mybir.ActivationFunctionType.Ln)
                nc.scalar.activation(rs16[DK:DK + 1, h, :],
                                     rs32[DK:DK + 1, h, :],
                                     mybir.ActivationFunctionType.Exp,
                                     scale=-1.0)
            muls = [(c01[0:DK, q0:q0 + QB], pvs[0]), (None, pvs[1]),
                    (ctx2[0:DK, q0:q0 + QB], pvs[2])]
            for h, (dst, pv) in enumerate(muls):
                bc = psA.tile([128, QB], F32, tag="A", name=f"bc{i}_{h}")
                nc.tensor.matmul(bc[0:DK, :], ones16[DK:DK + 1, 0:DK],
                                 rs16[DK:DK + 1, h, :], start=True, stop=True)
                bcs = c1s_pool.tile([DK, QB], F32, tag="bcs", name=f"bcs{i}_{h}")
                nc.vector.tensor_copy(bcs[:], bc[0:DK, :])
                if h == 1:
                    c1s = c1s_pool.tile([DK, QB], F16, tag="c1s",
                                        name=f"c1s{i}")
                    nc.vector.tensor_mul(c1s[:], pv[0:DK, :], bcs[:])
                    nc.sync.dma_start(c01[DK:128, q0:q0 + QB], c1s[:])
                else:
                    nc.vector.tensor_mul(dst, pv[0:DK, :], bcs[:])

        def outproj_chunk(chunk):
            csl = slice(chunk * 128, (chunk + 1) * 128)
            osb = osb_pool.tile([128, D], F16, tag="osb", name=f"osb{chunk}")
            for nb, ncols in ((0, 512), (512, 256)):
                po = psA.tile([128, QB], F32, tag="A", name=f"po{chunk}_{nb}")
                nc.tensor.matmul(po[:, 0:ncols], c01[:, csl],
                                 wo2_sb[:, nb:nb + ncols],
                                 start=True, stop=False)
                nc.tensor.matmul(po[:, 0:ncols], ctx2[0:DK, csl],
                                 wo3_sb[:, nb:nb + ncols],
                                 start=False, stop=True)
                nc.vector.tensor_copy(osb[:, nb:nb + ncols], po[:, 0:ncols])
            nc.sync.dma_start(o16[csl, :], osb[:])

        # ---- explicit schedule ------------------------------------------
        # Keep the ACT exp stream dense from ~12us on: QK of each block is
        # woven between projection/V chains and earlier blocks' PV work.
        proj_chain(1, 0)              # kT01 cols 0-512
        proj_chain(0, 0)              # qT01 cols 0-512
        qk01(0, 0); qk01(0, 1)        # exps(0) h0/h1
        for sb in range(NQB):         # [h2q|h2k] projection
            proj_chain(2, sb)
        # partition-swapped copy of [h2q|h2k] for paired h2 QK^T
        nc.sync.dma_start(qkT2b[0:DK, :], qkT2a[DK:128, :])
        nc.sync.dma_start(qkT2b[DK:128, :], qkT2a[0:DK, :])
        qk2(0, 0); qk2(0, 1)          # exps(0) h2
        proj_chain(1, 1); proj_chain(0, 1)
        for g2 in range(4):           # exps(1) h0/h1
            qk01(1, g2)
        for st in range(4):           # V chunks for PV(0)
            v_chain(st)
        mask_pv(0)
        for g2 in range(4):           # exps(1) h2
            qk2(1, g2)
        recip_bcmul(0)
        proj_chain(1, 2); proj_chain(0, 2)
        fill = [0, 1, 2, 3]
        for g2 in range(6):           # exps(2) h0/h1 + block-0 out-proj
            qk01(2, g2)
            if fill:
                outproj_chunk(fill.pop(0))
        for st in range(4, 8):
            v_chain(st)
        mask_pv(1)
        for g2 in range(6):           # exps(2) h2
            qk2(2, g2)
        recip_bcmul(1)
        proj_chain(1, 3); proj_chain(0, 3)
        fill = [4, 5, 6, 7]
        for g2 in range(8):           # exps(3) h0/h1 + block-1 out-proj
            qk01(3, g2)
            if fill:
                outproj_chunk(fill.pop(0))
        for st in range(8, 12):       # V for PV(2) + exps(3) h2 interleaved
            v_chain(st)
            qk2(3, st - 8)
        mask_pv(2)
        for st in range(12, 16):
            v_chain(st)
            qk2(3, st - 8)
        recip_bcmul(2)
        for chunk in range(8, 12):    # block-2 out-proj
            outproj_chunk(chunk)
        mask_pv(3)
        recip_bcmul(3)
        for chunk in range(12, 16):
            outproj_chunk(chunk)

        for p in (osb_pool, c1s_pool, psC, psA, psA2, cst):
            p.release()

    split_multi_waits(nc)
    return nc


def _core_inputs_fast(x, wq, bq, wk, bk, wv, bv, wo, with_qk_bias):
    wq8 = (wq * 0.125).astype(np.float32)
    bq8 = (bq * 0.125).astype(np.float32)
    ins = []
    for core in range(N_CORES):
        b, g = divmod(core, 4)
        hs = [HPC * g + k for k in range(HPC)]
        cols = lambda w, h: w[:, h * DK:(h + 1) * DK]
        seg = lambda v, h: v[h * DK:(h + 1) * DK]

        xT = np.ascontiguousarray(x[b].T).reshape(DKT, 128, S)
        xT = np.ascontiguousarray(xT.transpose(1, 0, 2)).astype(np.float16)

        w3 = np.stack([
            np.concatenate([cols(wq8, hs[0]), cols(wq8, hs[1])], axis=1),
            np.concatenate([cols(wk, hs[0]), cols(wk, hs[1])], axis=1),
            np.concatenate([cols(wq8, hs[2]), cols(wk, hs[2])], axis=1),
        ], axis=1)  # [768, 3, 128]
        wqk = np.ascontiguousarray(
            w3.reshape(DKT, 128, 3, 128).transpose(1, 0, 2, 3)).astype(np.float16)

        wv3 = np.concatenate([cols(wv, h) for h in hs], axis=1)
        wv3 = np.ascontiguousarray(
            wv3.reshape(DKT, 128, HPC * DK).transpose(1, 0, 2)).astype(np.float16)

        wo2 = np.concatenate([wo[hs[0] * DK:(hs[0] + 1) * DK],
                              wo[hs[1] * DK:(hs[1] + 1) * DK]]).astype(np.float16)
        wo3 = wo[hs[2] * DK:(hs[2] + 1) * DK].astype(np.float16)

        m = {"xT": xT, "wqk": wqk, "wv3": wv3, "wo2": wo2, "wo3": wo3}
        if with_qk_bias:
            bqk3 = np.stack([
                np.concatenate([seg(bq8, hs[0]), seg(bq8, hs[1])]),
                np.concatenate([seg(bk, hs[0]), seg(bk, hs[1])]),
                np.concatenate([seg(bq8, hs[2]), seg(bk, hs[2])]),
            ], axis=1).astype(np.float32)
            m["bqk3"] = bqk3
        ins.append(m)
    return ins


# ---------------------------------------------------------------------------
# fallback kernel (arbitrary mask): mask multiply path from the baseline


def build_nc_fallback():
    nc = bass.Bass("TRN2", target_bir_lowering=False, debug=False,
                   num_devices=N_CORES)

    x16 = nc.dram_tensor("x16", (S, D), F16, kind="ExternalInput").ap()
    wqe = nc.dram_tensor("wqe", (D, 256), F16, kind="ExternalInput").ap()
    wke = nc.dram_tensor("wke", (D, 256), F16, kind="ExternalInput").ap()
    wve = nc.dram_tensor("wve", (D, HPC * DK), F16, kind="ExternalInput").ap()
    woe = nc.dram_tensor("woe", (DK, HPC, D), F16, kind="ExternalInput").ap()
    woe2 = nc.dram_tensor("woe2", (128, D), F16, kind="ExternalInput").ap()
    bqk = nc.dram_tensor("bqk", (128, 4), F32, kind="ExternalInput").ap()
    bvp = nc.dram_tensor("bvp", (DK, HPC), F32, kind="ExternalInput").ap()
    mTd = nc.dram_tensor("mT", (S, S), F16, kind="ExternalInput").ap()
    o16 = nc.dram_tensor("o16", (S, D), F16, kind="ExternalOutput").ap()

    with tile.TileContext(nc) as tc:
        cst = tc.alloc_tile_pool(name="cst", bufs=1)
        ps_gen = tc.alloc_tile_pool(name="psg", bufs=2, space="PSUM")
        ps_sc = tc.alloc_tile_pool(name="pss", bufs=2, space="PSUM")
        ps_pv = tc.alloc_tile_pool(name="psv", bufs=2, space="PSUM")
        pt_pool = tc.alloc_tile_pool(name="ptp", bufs=2)
        ctx_pool = tc.alloc_tile_pool(name="ctp", bufs=2)
        rs_pool = tc.alloc_tile_pool(name="rsp", bufs=2)
        out_pool = tc.alloc_tile_pool(name="outp", bufs=3)
        msk_pool = tc.alloc_tile_pool(name="mskp", bufs=2)

        xT = cst.tile([128, DKT, S], F16, tag="xT")
        for c in range(DKT):
            nc.sync.dma_start_transpose(xT[:, c, :], x16[:, c * 128:(c + 1) * 128])

        wq_sb = cst.tile([128, DKT, 256], F16, tag="wq")
        wk_sb = cst.tile([128, DKT, 256], F16, tag="wk")
        wv_sb = cst.tile([128, DKT, HPC * DK], F16, tag="wv")
        wo_sb = cst.tile([DK, HPC, D], F16, tag="wo")
        wo_sb2 = cst.tile([128, D], F16, tag="wo2")
        nc.sync.dma_start(wq_sb[:], wqe.rearrange("(kt p) m -> p kt m", p=128))
        nc.sync.dma_start(wk_sb[:], wke.rearrange("(kt p) m -> p kt m", p=128))
        nc.sync.dma_start(wv_sb[:], wve.rearrange("(kt p) m -> p kt m", p=128))
        nc.sync.dma_start(wo_sb[:], woe)
        nc.sync.dma_start(wo_sb2[:], woe2)
        bqk_sb = cst.tile([128, 4], F32, tag="bqk")
        bv_sb = cst.tile([DK, HPC], F32, tag="bv")
        nc.sync.dma_start(bqk_sb[:], bqk)
        nc.sync.dma_start(bv_sb[:], bvp)

        ones = cst.tile([128, DK], F32, tag="ones")
        nc.vector.memset(ones[:], 1.0)

        V_sb = cst.tile([128, NKT, HPC, DK + 1], F16, tag="V")
        nc.vector.memset(V_sb[:], 1.0)  # pre-sets the rowsum ones-column

        qT0 = cst.tile([128, S], F16, tag="qT0")
        qT1 = cst.tile([128, S], F16, tag="qT1")
        kT0 = cst.tile([128, S], F16, tag="kT0")
        kT1 = cst.tile([128, S], F16, tag="kT1")

        for wsb, dsts, bcol in ((wq_sb, (qT0, qT1), 0), (wk_sb, (kT0, kT1), 2)):
            for t in range(2):
                for sb in range(NQB):
                    ps = ps_gen.tile([128, QB], F32, tag="psg", name=f"psp{t}{sb}")
                    for c in range(DKT):
                        nc.tensor.matmul(
                            ps[:], wsb[:, c, t * 128:(t + 1) * 128],
                            xT[:, c, sb * QB:(sb + 1) * QB],
                            start=(c == 0), stop=(c == DKT - 1))
                    nc.vector.tensor_scalar_add(
                        dsts[t][:, sb * QB:(sb + 1) * QB], ps[:],
                        bqk_sb[:, bcol + t:bcol + t + 1])

        for st in range(NKT):
            ps = ps_gen.tile([128, QB], F32, tag="psg", name=f"psv{st}")
            for c in range(DKT):
                nc.tensor.matmul(
                    ps[:, 0:HPC * DK], xT[:, c, st * 128:(st + 1) * 128],
                    wv_sb[:, c, :], start=(c == 0), stop=(c == DKT - 1))
            nc.vector.tensor_copy(
                V_sb[:, st, :, 0:DK],
                ps[:, 0:HPC * DK].rearrange("p (h d) -> p h d", d=DK))

        for i in range(NQB):
            ctx = ctx_pool.tile([DK, HPC, QB], F16, tag="ctx")
            c01 = ctx_pool.tile([128, QB], F16, tag="c01")
            mtile = msk_pool.tile([128, NKT, QB], F16, tag="mt")
            nc.sync.dma_start(
                mtile[:],
                mTd.rearrange("(kt p) q -> p kt q", p=128)[:, :, i * QB:(i + 1) * QB])
        # (kept identical to the baseline fallback loop below)
            kt = NKT
            qcols = slice(i * QB, (i + 1) * QB)
            pts = [pt_pool.tile([128, NKT, QB], F16, tag=f"pt{h}",
                                name=f"pt{h}") for h in range(HPC)]
            for g2 in range(kt // 2):
                scA = ps_sc.tile([128, 2, QB], F32, tag="sc", name="scA")
                scB = ps_sc.tile([128, 2, QB], F32, tag="sc", name="scB")
                for jj in range(2):
                    j = 2 * g2 + jj
                    nc.tensor.matmul(
                        scA[:, jj, :], kT0[0:DK, j * 128:(j + 1) * 128],
                        qT0[0:DK, qcols], start=True, stop=True,
                        tile_position=(0, 0))
                    nc.tensor.matmul(
                        scB[:, jj, :], kT0[DK:128, j * 128:(j + 1) * 128],
                        qT0[DK:128, qcols], start=True, stop=True,
                        tile_position=(DK, 0))
                nc.scalar.activation(pts[0][:, 2 * g2:2 * g2 + 2, :], scA[:],
                                     mybir.ActivationFunctionType.Exp)
                nc.scalar.activation(pts[1][:, 2 * g2:2 * g2 + 2, :], scB[:],
                                     mybir.ActivationFunctionType.Exp)
            for g2 in range(kt // 2):
                scC = ps_sc.tile([128, 2, QB], F32, tag="sc", name="scC")
                j0, j1 = 2 * g2, 2 * g2 + 1
                nc.tensor.matmul(
                    scC[:, 0, :], kT1[0:DK, j0 * 128:(j0 + 1) * 128],
                    qT1[0:DK, qcols], start=True, stop=True,
                    tile_position=(0, 0))
                nc.tensor.matmul(
                    scC[:, 1, :], kT1[DK:128, j1 * 128:(j1 + 1) * 128],
                    qT1[DK:128, qcols], start=True, stop=True,
                    tile_position=(DK, 0))
                nc.scalar.activation(pts[2][:, 2 * g2:2 * g2 + 2, :], scC[:],
                                     mybir.ActivationFunctionType.Exp)
            for h in range(HPC):
                pt = pts[h]
                for j in range(NKT):
                    nc.vector.tensor_mul(
                        pt[:, j, :], pt[:, j, :], mtile[:, j, :])

                pv = ps_pv.tile([128, QB], F32, tag="pv", name="pv")
                for j in range(kt):
                    nc.tensor.matmul(
                        pv[0:DK + 1, :], V_sb[:, j, h, :],
                        pt[:, j, :],
                        start=(j == 0), stop=(j == kt - 1),
                        skip_group_check=True)

                dst = c01[0:DK, :] if h == 0 else ctx[:, h, :]
                rs = rs_pool.tile([128, QB], F32, tag="rs", name="rs")
                nc.vector.reciprocal(rs[DK:DK + 1, :], pv[DK:DK + 1, :])
                bc = ps_gen.tile([128, QB], F32, tag="psg", name="bc")
                nc.tensor.matmul(bc[0:DK, :], ones[DK:DK + 1, 0:DK],
                                 rs[DK:DK + 1, :], start=True, stop=True,
                                 tile_position=(DK, 0))
                bcs = rs_pool.tile([DK, QB], F32, tag="bcs", name="bcs")
                nc.vector.tensor_copy(bcs[:], bc[0:DK, :])
                nc.vector.tensor_mul(dst, pv[0:DK, :], bcs[:])
                nc.vector.tensor_scalar_add(dst, dst, bv_sb[:, h:h + 1])

            nc.sync.dma_start(c01[DK:128, :], ctx[:, 1, :])
            for cch in range(QB // 128):
                chunk = i * (QB // 128) + cch
                csl = slice(cch * 128, (cch + 1) * 128)
                osb = out_pool.tile([128, D], F16, tag="osb", name="osb")
                for nb, ncols in ((0, 512), (512, 256)):
                    ps = ps_gen.tile([128, QB], F32, tag="psg", name="pso")
                    nc.tensor.matmul(ps[:, 0:ncols], c01[:, csl],
                                     wo_sb2[:, nb:nb + ncols],
                                     start=True, stop=False)
                    nc.tensor.matmul(ps[:, 0:ncols], ctx[:, 2, csl],
                                     wo_sb[:, 2, nb:nb + ncols],
                                     start=False, stop=True)
                    nc.vector.tensor_copy(osb[:, nb:nb + ncols], ps[:, 0:ncols])
                nc.sync.dma_start(o16[chunk * 128:(chunk + 1) * 128, :], osb[:])

        pools = [cst, ps_gen, ps_sc, ps_pv, pt_pool, ctx_pool, rs_pool,
                 out_pool, msk_pool]
        for p in reversed(pools):
            p.release()

    split_multi_waits(nc)
    return nc


def _core_inputs_fallback(x, mask, wq, bq, wk, bk, wv, bv, wo):
    ins = []
    wq8 = (wq * 0.125).astype(np.float32)
    bq8 = (bq * 0.125).astype(np.float32)
    mT = (mask[0, 0].T != 0).astype(np.float16)
    for core in range(N_CORES):
        b, g = divmod(core, 4)
        hs = [HPC * g + k for k in range(HPC)]
        cols = lambda w, h: w[:, h * DK:(h + 1) * DK]

        wqe = np.concatenate(
            [cols(wq8, hs[0]), cols(wq8, hs[1]), cols(wq8, hs[2]), cols(wq8, hs[2])],
            axis=1).astype(np.float16)
        wke = np.concatenate(
            [cols(wk, hs[0]), cols(wk, hs[1]), cols(wk, hs[2]), cols(wk, hs[2])],
            axis=1).astype(np.float16)
        wve = np.concatenate([cols(wv, h) for h in hs], axis=1).astype(np.float16)
        woe = wo.reshape(H, DK, D)[hs].transpose(1, 0, 2).astype(np.float16)
        woe2 = np.concatenate([wo[hs[0] * DK:(hs[0] + 1) * DK],
                               wo[hs[1] * DK:(hs[1] + 1) * DK]]).astype(np.float16)
        seg = lambda v, h: v[h * DK:(h + 1) * DK]
        bqk_pack = np.stack([
            np.concatenate([seg(bq8, hs[0]), seg(bq8, hs[1])]),
            np.concatenate([seg(bq8, hs[2]), seg(bq8, hs[2])]),
            np.concatenate([seg(bk, hs[0]), seg(bk, hs[1])]),
            np.concatenate([seg(bk, hs[2]), seg(bk, hs[2])]),
        ], axis=1).astype(np.float32)
        bvp = np.stack([seg(bv, h) for h in hs], axis=1).astype(np.float32)

        ins.append({
            "x16": x[b].astype(np.float16),
            "wqe": wqe, "wke": wke, "wve": wve, "woe": woe, "woe2": woe2,
            "bqk": bqk_pack, "bvp": bvp, "mT": mT,
        })
    return ins


# ---------------------------------------------------------------------------

_CACHE = {}


def _get_nc(key):
    if key not in _CACHE:
        if key == "fallback":
            _CACHE[key] = build_nc_fallback()
        else:
            _CACHE[key] = build_nc_fast(with_qk_bias=(key == "fast_bias"))
    return _CACHE[key]


def _prep(x, mask, wq, bq, wk, bk, wv, bv, wo):
    """Returns (nc, per-core input maps, mode string)."""
    m2 = np.asarray(mask[0, 0])
    causal = bool(np.array_equal(m2, np.tril(np.ones((S, S), m2.dtype))))
    if causal:
        with_bias = bool(np.any(bq) or np.any(bk))
        key = "fast_bias" if with_bias else "fast"
        nc = _get_nc(key)
        ins = _core_inputs_fast(x, wq, bq, wk, bk, wv, bv, wo, with_bias)
        return nc, ins, key
    nc = _get_nc("fallback")
    ins = _core_inputs_fallback(x, mask, wq, bq, wk, bk, wv, bv, wo)
    return nc, ins, "fallback"


def kernel(x, mask, wq, bq, wk, bk, wv, bv, wo, bo):
    x = np.asarray(x)
    mask = np.asarray(mask)
    wq, bq = np.asarray(wq), np.asarray(bq)
    wk, bk = np.asarray(wk), np.asarray(bk)
    wv, bv = np.asarray(wv), np.asarray(bv)
    wo, bo = np.asarray(wo), np.asarray(bo)
    nc, ins, mode = _prep(x, mask, wq, bq, wk, bk, wv, bv, wo)
    res = run_bass_kernel_spmd(nc, ins, core_ids=list(range(N_CORES)))
    out = np.zeros((B, S, D), np.float32)
    for core in range(N_CORES):
        b = core // 4
        out[b] += res.results[core]["o16"].astype(np.float32)
    out += np.asarray(bo, np.float32)
    if mode != "fallback":
        # bv was folded out of the device kernel: ctx/r + 1*bv^T through wo
        out += bv.astype(np.float32) @ wo.astype(np.float32)
    return out


# revision 19
# speedup vs baseline: 1.1923x; 1.1923x over previous
"""Multi-head causal attention (B=2,S=2048,D=768,H=12) on 8 NeuronCores.

Sharding: core = (batch, head_group) with 2 batches x 4 head groups of 3
heads.  Each core computes q/k/v projections for its heads, causal
attention, and a partial output projection (wo rows for its heads); the
host sums the 4 partials per batch and adds bo (+ folded bv @ wo).

Fast path (causal mask, zero q/k biases):
  - x is pre-transposed on the host; all loads are plain contiguous DMAs
    issued weights-first so the first matmul starts ~4us in.
  - Q/K projections packed into 3 M-tiles: [h0q|h1q], [h0k|h1k],
    [h2q|h2k]; a partition-swapped copy of the last enables row-paired
    h2 QK^T matmuls.
  - QK^T row-paired (two 64-contraction matmuls in disjoint PE row
    groups run concurrently); diagonal 128-blocks are column-trimmed in
    QK^T, exp, and PV.
  - softmax denominators: ones-column in V (free rowsum), fast DVE
    reciprocal, fp16 PE broadcast, one DVE multiply per head.
  - causal tril masking on GpSimd; PSUM time-shared 4/2/2 banks.
"""

import numpy as np

import bass_rust
import concourse.bass as bass
import concourse.mybir as mybir
import concourse.tile as tile
from concourse.bass_utils import run_bass_kernel_spmd

F16 = mybir.dt.float16
F32 = mybir.dt.float32

B, S, D = 2, 2048, 768
H, DK = 12, 64
HPC = 3            # heads per core
N_CORES = 8
QB = 512           # query block (psum free dim)
NQB = S // QB      # 4
NKT = S // 128     # 16 key tiles
DKT = D // 128     # 6 contraction tiles for projections

ScopedClock = bass_rust.ScopedClock


# ---------------------------------------------------------------------------
# walrus in this build accepts at most ONE sync-wait per instruction; spread
# extra waits onto NOPs placed immediately before the owning instruction.

def _split_drain_and_barrier(self, tick_clock, wait_clock):
    probe = self.nc.sync.nop()
    wait_clock.add_sem_waits(probe.ins, ScopedClock({None: tick_clock.global_clock}))
    si = probe.ins.sync_info
    waits = list(si.on_wait) if si is not None else []
    if len(waits) > 1:
        si.on_wait = waits[:1]
        for w in waits[1:]:
            n = self.nc.sync.nop()
            nsi = n.ins.sync_info
            if nsi is None:
                n.ins.sync_info = bass_rust.SyncInfo(on_wait=[w], on_update=[])
            else:
                nsi.on_wait = [w]
    self.nc.sync.drain()

    self.nc.all_engine_barrier()
    assert self.sems is not None
    popped = self.nc._tile_sem_poison_stack.pop()
    assert popped is self._sem_poison
    self.nc.clear_and_free_semaphores(list(self.sems.allocated().values()))
    self.nc.all_engine_barrier()


tile.TileContext._drain_and_barrier = _split_drain_and_barrier

_nop_ctr = [0]


def split_multi_waits(nc):
    def visit(parent):
        for bb in parent.blocks:
            insts = bb.instructions
            out = []
            changed = False
            for inst in insts:
                si = inst.sync_info
                if si is not None and len(si.on_wait) > 1:
                    waits = list(si.on_wait)
                    for w in waits[:-1]:
                        _nop_ctr[0] += 1
                        nop = mybir.InstNoOp(
                            name=f"wsplit{_nop_ctr[0]}",
                            sync_info=mybir.SyncInfo(on_wait=[w], on_update=[]),
                            bass_nofuse=True,
                            engine=inst.engine,
                        )
                        out.append(nop)
                    si.on_wait = waits[-1:]
                    changed = True
                out.append(inst)
            if changed:
                bb.instructions = out
    for f in nc.m.functions:
        visit(f)


# ---------------------------------------------------------------------------
# fast causal kernel


def build_nc_fast(with_qk_bias: bool = False):
    nc = bass.Bass("TRN2", target_bir_lowering=False, debug=False,
                   num_devices=N_CORES)

    xTd = nc.dram_tensor("xT", (128, DKT, S), F16, kind="ExternalInput").ap()
    wqkd = nc.dram_tensor("wqk", (128, DKT, 3, 128), F16, kind="ExternalInput").ap()
    wvd = nc.dram_tensor("wv3", (128, DKT, HPC * DK), F16, kind="ExternalInput").ap()
    wo2d = nc.dram_tensor("wo2", (128, D), F16, kind="ExternalInput").ap()
    wo3d = nc.dram_tensor("wo3", (DK, D), F16, kind="ExternalInput").ap()
    if with_qk_bias:
        bqkd = nc.dram_tensor("bqk3", (128, 3), F32, kind="ExternalInput").ap()
    o16 = nc.dram_tensor("o16", (S, D), F16, kind="ExternalOutput").ap()

    with tile.TileContext(nc) as tc:
        cst = tc.alloc_tile_pool(name="cst", bufs=1)
        psA2 = tc.alloc_tile_pool(name="psA2", bufs=2, space="PSUM")  # 4 banks
        psA = tc.alloc_tile_pool(name="psA", bufs=2, space="PSUM")    # 2 banks
        psC = tc.alloc_tile_pool(name="psC", bufs=2, space="PSUM")    # 2 banks
        c1s_pool = tc.alloc_tile_pool(name="c1s", bufs=2)
        osb_pool = tc.alloc_tile_pool(name="osbp", bufs=3)

        # ---- constants / weights first (small), then x chunks
        wqk_sb = cst.tile([128, DKT, 3, 128], F16, tag="wqk")
        wv_sb = cst.tile([128, DKT, HPC * DK], F16, tag="wv")
        wo2_sb = cst.tile([128, D], F16, tag="wo2")
        wo3_sb = cst.tile([DK, D], F16, tag="wo3")
        for m in (1, 0, 2):
            nc.sync.dma_start(wqk_sb[:, :, m, :], wqkd[:, :, m, :])
        nc.sync.dma_start(wv_sb[:], wvd)
        nc.sync.dma_start(wo2_sb[:], wo2d)
        nc.sync.dma_start(wo3_sb[:], wo3d)
        if with_qk_bias:
            bqk_sb = cst.tile([128, 3], F32, tag="bqk")
            nc.sync.dma_start(bqk_sb[:], bqkd)

        xt = cst.tile([128, DKT, S], F16, tag="xt")
        for c in range(DKT):
            eng = nc.scalar if c % 2 == 0 else nc.gpsimd
            eng.dma_start(xt[:, c, :], xTd[:, c, :])

        tril = cst.tile([128, 128], F16, tag="tril")
        nc.gpsimd.memset(tril[:], 1.0)
        # keep (f - p >= 0) i.e. q >= k, else 0
        nc.gpsimd.affine_select(
            out=tril[:], in_=tril[:], compare_op=mybir.AluOpType.is_ge,
            fill=0.0, base=0, pattern=[[1, 128]], channel_multiplier=-1)

        ones16 = cst.tile([128, DK], F16, tag="ones16")
        nc.vector.memset(ones16[:], 1.0)

        V_sb = cst.tile([128, NKT, HPC, DK + 1], F16, tag="V")
        nc.gpsimd.memset(V_sb[:, :, :, DK:DK + 1], 1.0)  # rowsum ones-column

        qT01 = cst.tile([128, S], F16, tag="qT01")
        kT01 = cst.tile([128, S], F16, tag="kT01")
        qkT2a = cst.tile([128, S], F16, tag="qkT2a")
        qkT2b = cst.tile([128, S], F16, tag="qkT2b")

        # double-buffered per i-parity so exp(i+1) overlaps PV(i)
        pts2 = [[cst.tile([128, NKT, QB], F16, tag=f"pt{h}_{par}",
                          name=f"pt{h}_{par}") for h in range(HPC)]
                for par in range(2)]

        c01 = cst.tile([128, S], F16, tag="c01")
        ctx2 = cst.tile([128, S], F16, tag="ctx2")
        rs32 = cst.tile([128, HPC, QB], F32, tag="rs32")
        rs16 = cst.tile([128, HPC, QB], F16, tag="rs16")

        # ---- emission helpers -------------------------------------------
        dsts = (qT01, kT01, qkT2a)

        def proj_chain(m, sb):
            pp = psA.tile([128, QB], F32, tag="A", name=f"pp{m}_{sb}")
            for c in range(DKT):
                nc.tensor.matmul(
                    pp[:], wqk_sb[:, c, m, :],
                    xt[:, c, sb * QB:(sb + 1) * QB],
                    start=(c == 0), stop=(c == DKT - 1))
            dcols = dsts[m][:, sb * QB:(sb + 1) * QB]
            if with_qk_bias:
                nc.vector.tensor_scalar_add(dcols, pp[:], bqk_sb[:, m:m + 1])
            else:
                nc.vector.tensor_copy(dcols, pp[:])

        def v_chain(st):
            pp = psA.tile([128, QB], F32, tag="A", name=f"ppv{st}")
            for c in range(DKT):
                nc.tensor.matmul(
                    pp[:, 0:HPC * DK], xt[:, c, st * 128:(st + 1) * 128],
                    wv_sb[:, c, :], start=(c == 0), stop=(c == DKT - 1))
            nc.vector.tensor_copy(
                V_sb[:, st, :, 0:DK],
                pp[:, 0:HPC * DK].rearrange("p (h d) -> p h d", d=DK))

        def off_of(i, j):
            return 128 * (j - 4 * i) if j >= 4 * i else 0

        def qk01(i, g2):
            q0 = i * QB
            pt0, pt1 = pts2[i % 2][0], pts2[i % 2][1]
            j0, j1 = 2 * g2, 2 * g2 + 1
            off0 = off_of(i, j0)
            sA = psA2.tile([128, 2, QB], F32, tag="A2", name=f"sA{i}_{g2}")
            sB = psA2.tile([128, 2, QB], F32, tag="A2", name=f"sB{i}_{g2}")
            for jj, j in ((0, j0), (1, j1)):
                off = off_of(i, j)
                nc.tensor.matmul(
                    sA[:, jj, off:], kT01[0:DK, j * 128:(j + 1) * 128],
                    qT01[0:DK, q0 + off:q0 + QB], start=True, stop=True)
                nc.tensor.matmul(
                    sB[:, jj, off:], kT01[DK:128, j * 128:(j + 1) * 128],
                    qT01[DK:128, q0 + off:q0 + QB], start=True, stop=True)
            # one exp per [128,2,QB] tile; cols < off1 of plane j1 are
            # garbage-but-never-read (PV/tril only touch cols >= off)
            nc.scalar.activation(pt0[:, j0:j0 + 2, off0:], sA[:, :, off0:],
                                 mybir.ActivationFunctionType.Exp)
            nc.scalar.activation(pt1[:, j0:j0 + 2, off0:], sB[:, :, off0:],
                                 mybir.ActivationFunctionType.Exp)

        def qk2(i, g2):
            q0 = i * QB
            pt2 = pts2[i % 2][2]
            j0, j1 = 2 * g2, 2 * g2 + 1
            off0, off1 = off_of(i, j0), off_of(i, j1)
            sC = psA2.tile([128, 2, QB], F32, tag="A2", name=f"sC{i}_{g2}")
            nc.tensor.matmul(
                sC[:, 0, off0:], qkT2b[0:DK, j0 * 128:(j0 + 1) * 128],
                qkT2a[0:DK, q0 + off0:q0 + QB], start=True, stop=True)
            nc.tensor.matmul(
                sC[:, 1, off1:], qkT2a[DK:128, j1 * 128:(j1 + 1) * 128],
                qkT2b[DK:128, q0 + off1:q0 + QB], start=True, stop=True)
            nc.scalar.activation(pt2[:, j0:j0 + 2, off0:], sC[:, :, off0:],
                                 mybir.ActivationFunctionType.Exp)

        pvs_of = {}

        def mask_pv(i):
            """tril mask (GpSimd) + PV accumulation chains (PE)."""
            kt = 4 * (i + 1)
            pts = pts2[i % 2]
            for h in range(HPC):
                for jj in range(4):
                    j = 4 * i + jj
                    off = 128 * jj
                    nc.gpsimd.tensor_mul(
                        pts[h][:, j, off:off + 128],
                        pts[h][:, j, off:off + 128], tril[:])
            pvs = []
            for h in range(HPC):
                pv = psC.tile([128, QB], F32, tag="C", name=f"pv{i}_{h}")
                pvs.append(pv)
                for j in range(kt):
                    off = off_of(i, j)
                    nc.tensor.matmul(
                        pv[0:DK + 1, off:QB], V_sb[:, j, h, :],
                        pts[h][:, j, off:QB],
                        start=(j == 0), stop=(j == kt - 1),
                        skip_group_check=True)
            pvs_of[i] = pvs

        def recip_bcmul(i):
            """1/r = exp(-ln r) on ACT (same table as the exp stream), PE
            broadcast, SBUF stage, DVE normalize multiply."""
            q0 = i * QB
            pvs = pvs_of[i]
            for h in range(HPC):
                nc.scalar.activation(rs32[DK:DK + 1, h, :],
                                     pvs[h][DK:DK + 1, :],
                                     mybir.ActivationFunctionType.Ln)
                nc.scalar.activation(rs16[DK:DK + 1, h, :],
                                     rs32[DK:DK + 1, h, :],
                                     mybir.ActivationFunctionType.Exp,
                                     scale=-1.0)
            muls = [(c01[0:DK, q0:q0 + QB], pvs[0]), (None, pvs[1]),
                    (ctx2[0:DK, q0:q0 + QB], pvs[2])]
            for h, (dst, pv) in enumerate(muls):
                bc = psA.tile([128, QB], F32, tag="A", name=f"bc{i}_{h}")
                nc.tensor.matmul(bc[0:DK, :], ones16[DK:DK + 1, 0:DK],
                                 rs16[DK:DK + 1, h, :], start=True, stop=True)
                bcs = c1s_pool.tile([DK, QB], F32, tag="bcs", name=f"bcs{i}_{h}")
                nc.vector.tensor_copy(bcs[:], bc[0:DK, :])
                if h == 1:
                    c1s = c1s_pool.tile([DK, QB], F16, tag="c1s",
                                        name=f"c1s{i}")
                    nc.vector.tensor_mul(c1s[:], pv[0:DK, :], bcs[:])
                    nc.sync.dma_start(c01[DK:128, q0:q0 + QB], c1s[:])
                else:
                    nc.vector.tensor_mul(dst, pv[0:DK, :], bcs[:])

        def outproj_chunk(chunk):
            csl = slice(chunk * 128, (chunk + 1) * 128)
            osb = osb_pool.tile([128, D], F16, tag="osb", name=f"osb{chunk}")
            for nb, ncols in ((0, 512), (512, 256)):
                po = psA.tile([128, QB], F32, tag="A", name=f"po{chunk}_{nb}")
                nc.tensor.matmul(po[:, 0:ncols], c01[:, csl],
                                 wo2_sb[:, nb:nb + ncols],
                                 start=True, stop=False)
                nc.tensor.matmul(po[:, 0:ncols], ctx2[0:DK, csl],
                                 wo3_sb[:, nb:nb + ncols],
                                 start=False, stop=True)
                nc.vector.tensor_copy(osb[:, nb:nb + ncols], po[:, 0:ncols])
            nc.sync.dma_start(o16[csl, :], osb[:])

        # ---- schedule: projections, V, then per-block attention with the
        # previous block's output projection interleaved into the QK stream
        for m in range(3):
            for sb in range(NQB):
                proj_chain(m, sb)
        for st in range(NKT):
            v_chain(st)
        # partition-swapped copy of [h2q|h2k] for paired h2 QK^T
        nc.sync.dma_start(qkT2b[0:DK, :], qkT2a[DK:128, :])
        nc.sync.dma_start(qkT2b[DK:128, :], qkT2a[0:DK, :])

        for i in range(NQB):
            fill = [4 * (i - 1) + c for c in range(4)] if i > 0 else []
            for g2 in range(2 * (i + 1)):
                qk01(i, g2)
                if fill:
                    outproj_chunk(fill.pop(0))
            for g2 in range(2 * (i + 1)):
                qk2(i, g2)
                if fill:
                    outproj_chunk(fill.pop(0))
            mask_pv(i)
            recip_bcmul(i)
        for chunk in range(12, 16):
            outproj_chunk(chunk)

        for p in (osb_pool, c1s_pool, psC, psA, psA2, cst):
            p.release()

    split_multi_waits(nc)
    return nc


def _core_inputs_fast(x, wq, bq, wk, bk, wv, bv, wo, with_qk_bias):
    wq8 = (wq * 0.125).astype(np.float32)
    bq8 = (bq * 0.125).astype(np.float32)
    ins = []
    for core in range(N_CORES):
        b, g = divmod(core, 4)
        hs = [HPC * g + k for k in range(HPC)]
        cols = lambda w, h: w[:, h * DK:(h + 1) * DK]
        seg = lambda v, h: v[h * DK:(h + 1) * DK]

        xT = np.ascontiguousarray(x[b].T).reshape(DKT, 128, S)
        xT = np.ascontiguousarray(xT.transpose(1, 0, 2)).astype(np.float16)

        w3 = np.stack([
            np.concatenate([cols(wq8, hs[0]), cols(wq8, hs[1])], axis=1),
            np.concatenate([cols(wk, hs[0]), cols(wk, hs[1])], axis=1),
            np.concatenate([cols(wq8, hs[2]), cols(wk, hs[2])], axis=1),
        ], axis=1)  # [768, 3, 128]
        wqk = np.ascontiguousarray(
            w3.reshape(DKT, 128, 3, 128).transpose(1, 0, 2, 3)).astype(np.float16)

        wv3 = np.concatenate([cols(wv, h) for h in hs], axis=1)
        wv3 = np.ascontiguousarray(
            wv3.reshape(DKT, 128, HPC * DK).transpose(1, 0, 2)).astype(np.float16)

        wo2 = np.concatenate([wo[hs[0] * DK:(hs[0] + 1) * DK],
                              wo[hs[1] * DK:(hs[1] + 1) * DK]]).astype(np.float16)
        wo3 = wo[hs[2] * DK:(hs[2] + 1) * DK].astype(np.float16)

        m = {"xT": xT, "wqk": wqk, "wv3": wv3, "wo2": wo2, "wo3": wo3}
        if with_qk_bias:
            bqk3 = np.stack([
                np.concatenate([seg(bq8, hs[0]), seg(bq8, hs[1])]),
                np.concatenate([seg(bk, hs[0]), seg(bk, hs[1])]),
                np.concatenate([seg(bq8, hs[2]), seg(bk, hs[2])]),
            ], axis=1).astype(np.float32)
            m["bqk3"] = bqk3
        ins.append(m)
    return ins


# ---------------------------------------------------------------------------
# fallback kernel (arbitrary mask): mask multiply path from the baseline


def build_nc_fallback():
    nc = bass.Bass("TRN2", target_bir_lowering=False, debug=False,
                   num_devices=N_CORES)

    x16 = nc.dram_tensor("x16", (S, D), F16, kind="ExternalInput").ap()
    wqe = nc.dram_tensor("wqe", (D, 256), F16, kind="ExternalInput").ap()
    wke = nc.dram_tensor("wke", (D, 256), F16, kind="ExternalInput").ap()
    wve = nc.dram_tensor("wve", (D, HPC * DK), F16, kind="ExternalInput").ap()
    woe = nc.dram_tensor("woe", (DK, HPC, D), F16, kind="ExternalInput").ap()
    woe2 = nc.dram_tensor("woe2", (128, D), F16, kind="ExternalInput").ap()
    bqk = nc.dram_tensor("bqk", (128, 4), F32, kind="ExternalInput").ap()
    bvp = nc.dram_tensor("bvp", (DK, HPC), F32, kind="ExternalInput").ap()
    mTd = nc.dram_tensor("mT", (S, S), F16, kind="ExternalInput").ap()
    o16 = nc.dram_tensor("o16", (S, D), F16, kind="ExternalOutput").ap()

    with tile.TileContext(nc) as tc:
        cst = tc.alloc_tile_pool(name="cst", bufs=1)
        ps_gen = tc.alloc_tile_pool(name="psg", bufs=2, space="PSUM")
        ps_sc = tc.alloc_tile_pool(name="pss", bufs=2, space="PSUM")
        ps_pv = tc.alloc_tile_pool(name="psv", bufs=2, space="PSUM")
        pt_pool = tc.alloc_tile_pool(name="ptp", bufs=2)
        ctx_pool = tc.alloc_tile_pool(name="ctp", bufs=2)
        rs_pool = tc.alloc_tile_pool(name="rsp", bufs=2)
        out_pool = tc.alloc_tile_pool(name="outp", bufs=3)
        msk_pool = tc.alloc_tile_pool(name="mskp", bufs=2)

        xT = cst.tile([128, DKT, S], F16, tag="xT")
        for c in range(DKT):
            nc.sync.dma_start_transpose(xT[:, c, :], x16[:, c * 128:(c + 1) * 128])

        wq_sb = cst.tile([128, DKT, 256], F16, tag="wq")
        wk_sb = cst.tile([128, DKT, 256], F16, tag="wk")
        wv_sb = cst.tile([128, DKT, HPC * DK], F16, tag="wv")
        wo_sb = cst.tile([DK, HPC, D], F16, tag="wo")
        wo_sb2 = cst.tile([128, D], F16, tag="wo2")
        nc.sync.dma_start(wq_sb[:], wqe.rearrange("(kt p) m -> p kt m", p=128))
        nc.sync.dma_start(wk_sb[:], wke.rearrange("(kt p) m -> p kt m", p=128))
        nc.sync.dma_start(wv_sb[:], wve.rearrange("(kt p) m -> p kt m", p=128))
        nc.sync.dma_start(wo_sb[:], woe)
        nc.sync.dma_start(wo_sb2[:], woe2)
        bqk_sb = cst.tile([128, 4], F32, tag="bqk")
        bv_sb = cst.tile([DK, HPC], F32, tag="bv")
        nc.sync.dma_start(bqk_sb[:], bqk)
        nc.sync.dma_start(bv_sb[:], bvp)

        ones = cst.tile([128, DK], F32, tag="ones")
        nc.vector.memset(ones[:], 1.0)

        V_sb = cst.tile([128, NKT, HPC, DK + 1], F16, tag="V")
        nc.vector.memset(V_sb[:], 1.0)  # pre-sets the rowsum ones-column

        qT0 = cst.tile([128, S], F16, tag="qT0")
        qT1 = cst.tile([128, S], F16, tag="qT1")
        kT0 = cst.tile([128, S], F16, tag="kT0")
        kT1 = cst.tile([128, S], F16, tag="kT1")

        for wsb, dsts, bcol in ((wq_sb, (qT0, qT1), 0), (wk_sb, (kT0, kT1), 2)):
            for t in range(2):
                for sb in range(NQB):
                    ps = ps_gen.tile([128, QB], F32, tag="psg", name=f"psp{t}{sb}")
                    for c in range(DKT):
                        nc.tensor.matmul(
                            ps[:], wsb[:, c, t * 128:(t + 1) * 128],
                            xT[:, c, sb * QB:(sb + 1) * QB],
                            start=(c == 0), stop=(c == DKT - 1))
                    nc.vector.tensor_scalar_add(
                        dsts[t][:, sb * QB:(sb + 1) * QB], ps[:],
                        bqk_sb[:, bcol + t:bcol + t + 1])

        for st in range(NKT):
            ps = ps_gen.tile([128, QB], F32, tag="psg", name=f"psv{st}")
            for c in range(DKT):
                nc.tensor.matmul(
                    ps[:, 0:HPC * DK], xT[:, c, st * 128:(st + 1) * 128],
                    wv_sb[:, c, :], start=(c == 0), stop=(c == DKT - 1))
            nc.vector.tensor_copy(
                V_sb[:, st, :, 0:DK],
                ps[:, 0:HPC * DK].rearrange("p (h d) -> p h d", d=DK))

        for i in range(NQB):
            ctx = ctx_pool.tile([DK, HPC, QB], F16, tag="ctx")
            c01 = ctx_pool.tile([128, QB], F16, tag="c01")
            mtile = msk_pool.tile([128, NKT, QB], F16, tag="mt")
            nc.sync.dma_start(
                mtile[:],
                mTd.rearrange("(kt p) q -> p kt q", p=128)[:, :, i * QB:(i + 1) * QB])
        # (kept identical to the baseline fallback loop below)
            kt = NKT
            qcols = slice(i * QB, (i + 1) * QB)
            pts = [pt_pool.tile([128, NKT, QB], F16, tag=f"pt{h}",
                                name=f"pt{h}") for h in range(HPC)]
            for g2 in range(kt // 2):
                scA = ps_sc.tile([128, 2, QB], F32, tag="sc", name="scA")
                scB = ps_sc.tile([128, 2, QB], F32, tag="sc", name="scB")
                for jj in range(2):
                    j = 2 * g2 + jj
                    nc.tensor.matmul(
                        scA[:, jj, :], kT0[0:DK, j * 128:(j + 1) * 128],
                        qT0[0:DK, qcols], start=True, stop=True,
                        tile_position=(0, 0))
                    nc.tensor.matmul(
                        scB[:, jj, :], kT0[DK:128, j * 128:(j + 1) * 128],
                        qT0[DK:128, qcols], start=True, stop=True,
                        tile_position=(DK, 0))
                nc.scalar.activation(pts[0][:, 2 * g2:2 * g2 + 2, :], scA[:],
                                     mybir.ActivationFunctionType.Exp)
                nc.scalar.activation(pts[1][:, 2 * g2:2 * g2 + 2, :], scB[:],
                                     mybir.ActivationFunctionType.Exp)
            for g2 in range(kt // 2):
                scC = ps_sc.tile([128, 2, QB], F32, tag="sc", name="scC")
                j0, j1 = 2 * g2, 2 * g2 + 1
                nc.tensor.matmul(
                    scC[:, 0, :], kT1[0:DK, j0 * 128:(j0 + 1) * 128],
                    qT1[0:DK, qcols], start=True, stop=True,
                    tile_position=(0, 0))
                nc.tensor.matmul(
                    scC[:, 1, :], kT1[DK:128, j1 * 128:(j1 + 1) * 128],
                    qT1[DK:128, qcols], start=True, stop=True,
                    tile_position=(DK, 0))
                nc.scalar.activation(pts[2][:, 2 * g2:2 * g2 + 2, :], scC[:],
                                     mybir.ActivationFunctionType.Exp)
            for h in range(HPC):
                pt = pts[h]
                for j in range(NKT):
                    nc.vector.tensor_mul(
                        pt[:, j, :], pt[:, j, :], mtile[:, j, :])

                pv = ps_pv.tile([128, QB], F32, tag="pv", name="pv")
                for j in range(kt):
                    nc.tensor.matmul(
                        pv[0:DK + 1, :], V_sb[:, j, h, :],
                        pt[:, j, :],
                        start=(j == 0), stop=(j == kt - 1),
                        skip_group_check=True)

                dst = c01[0:DK, :] if h == 0 else ctx[:, h, :]
                rs = rs_pool.tile([128, QB], F32, tag="rs", name="rs")
                nc.vector.reciprocal(rs[DK:DK + 1, :], pv[DK:DK + 1, :])
                bc = ps_gen.tile([128, QB], F32, tag="psg", name="bc")
                nc.tensor.matmul(bc[0:DK, :], ones[DK:DK + 1, 0:DK],
                                 rs[DK:DK + 1, :], start=True, stop=True,
                                 tile_position=(DK, 0))
                bcs = rs_pool.tile([DK, QB], F32, tag="bcs", name="bcs")
                nc.vector.tensor_copy(bcs[:], bc[0:DK, :])
                nc.vector.tensor_mul(dst, pv[0:DK, :], bcs[:])
                nc.vector.tensor_scalar_add(dst, dst, bv_sb[:, h:h + 1])

            nc.sync.dma_start(c01[DK:128, :], ctx[:, 1, :])
            for cch in range(QB // 128):
                chunk = i * (QB // 128) + cch
                csl = slice(cch * 128, (cch + 1) * 128)
                osb = out_pool.tile([128, D], F16, tag="osb", name="osb")
                for nb, ncols in ((0, 512), (512, 256)):
                    ps = ps_gen.tile([128, QB], F32, tag="psg", name="pso")
                    nc.tensor.matmul(ps[:, 0:ncols], c01[:, csl],
                                     wo_sb2[:, nb:nb + ncols],
                                     start=True, stop=False)
                    nc.tensor.matmul(ps[:, 0:ncols], ctx[:, 2, csl],
                                     wo_sb[:, 2, nb:nb + ncols],
                                     start=False, stop=True)
                    nc.vector.tensor_copy(osb[:, nb:nb + ncols], ps[:, 0:ncols])
                nc.sync.dma_start(o16[chunk * 128:(chunk + 1) * 128, :], osb[:])

        pools = [cst, ps_gen, ps_sc, ps_pv, pt_pool, ctx_pool, rs_pool,
                 out_pool, msk_pool]
        for p in reversed(pools):
            p.release()

    split_multi_waits(nc)
    return nc


def _core_inputs_fallback(x, mask, wq, bq, wk, bk, wv, bv, wo):
    ins = []
    wq8 = (wq * 0.125).astype(np.float32)
    bq8 = (bq * 0.125).astype(np.float32)
    mT = (mask[0, 0].T != 0).astype(np.float16)
    for core in range(N_CORES):
        b, g = divmod(core, 4)
        hs = [HPC * g + k for k in range(HPC)]
        cols = lambda w, h: w[:, h * DK:(h + 1) * DK]

        wqe = np.concatenate(
            [cols(wq8, hs[0]), cols(wq8, hs[1]), cols(wq8, hs[2]), cols(wq8, hs[2])],
            axis=1).astype(np.float16)
        wke = np.concatenate(
            [cols(wk, hs[0]), cols(wk, hs[1]), cols(wk, hs[2]), cols(wk, hs[2])],
            axis=1).astype(np.float16)
        wve = np.concatenate([cols(wv, h) for h in hs], axis=1).astype(np.float16)
        woe = wo.reshape(H, DK, D)[hs].transpose(1, 0, 2).astype(np.float16)
        woe2 = np.concatenate([wo[hs[0] * DK:(hs[0] + 1) * DK],
                               wo[hs[1] * DK:(hs[1] + 1) * DK]]).astype(np.float16)
        seg = lambda v, h: v[h * DK:(h + 1) * DK]
        bqk_pack = np.stack([
            np.concatenate([seg(bq8, hs[0]), seg(bq8, hs[1])]),
            np.concatenate([seg(bq8, hs[2]), seg(bq8, hs[2])]),
            np.concatenate([seg(bk, hs[0]), seg(bk, hs[1])]),
            np.concatenate([seg(bk, hs[2]), seg(bk, hs[2])]),
        ], axis=1).astype(np.float32)
        bvp = np.stack([seg(bv, h) for h in hs], axis=1).astype(np.float32)

        ins.append({
            "x16": x[b].astype(np.float16),
            "wqe": wqe, "wke": wke, "wve": wve, "woe": woe, "woe2": woe2,
            "bqk": bqk_pack, "bvp": bvp, "mT": mT,
        })
    return ins


# ---------------------------------------------------------------------------

_CACHE = {}


def _get_nc(key):
    if key not in _CACHE:
        if key == "fallback":
            _CACHE[key] = build_nc_fallback()
        else:
            _CACHE[key] = build_nc_fast(with_qk_bias=(key == "fast_bias"))
    return _CACHE[key]


def _prep(x, mask, wq, bq, wk, bk, wv, bv, wo):
    """Returns (nc, per-core input maps, mode string)."""
    m2 = np.asarray(mask[0, 0])
    causal = bool(np.array_equal(m2, np.tril(np.ones((S, S), m2.dtype))))
    if causal:
        with_bias = bool(np.any(bq) or np.any(bk))
        key = "fast_bias" if with_bias else "fast"
        nc = _get_nc(key)
        ins = _core_inputs_fast(x, wq, bq, wk, bk, wv, bv, wo, with_bias)
        return nc, ins, key
    nc = _get_nc("fallback")
    ins = _core_inputs_fallback(x, mask, wq, bq, wk, bk, wv, bv, wo)
    return nc, ins, "fallback"


def kernel(x, mask, wq, bq, wk, bk, wv, bv, wo, bo):
    x = np.asarray(x)
    mask = np.asarray(mask)
    wq, bq = np.asarray(wq), np.asarray(bq)
    wk, bk = np.asarray(wk), np.asarray(bk)
    wv, bv = np.asarray(wv), np.asarray(bv)
    wo, bo = np.asarray(wo), np.asarray(bo)
    nc, ins, mode = _prep(x, mask, wq, bq, wk, bk, wv, bv, wo)
    res = run_bass_kernel_spmd(nc, ins, core_ids=list(range(N_CORES)))
    out = np.zeros((B, S, D), np.float32)
    for core in range(N_CORES):
        b = core // 4
        out[b] += res.results[core]["o16"].astype(np.float32)
    out += np.asarray(bo, np.float32)
    if mode != "fallback":
        # bv was folded out of the device kernel: ctx/r + 1*bv^T through wo
        out += bv.astype(np.float32) @ wo.astype(np.float32)
    return out


# revision 23
# speedup vs baseline: 1.2394x; 1.0395x over previous
"""Multi-head causal attention (B=2,S=2048,D=768,H=12) on 8 NeuronCores.

Sharding: core = (batch, head_group) with 2 batches x 4 head groups of 3
heads.  Each core computes q/k/v projections for its heads, causal
attention, and a partial output projection (wo rows for its heads); the
host sums the 4 partials per batch and adds bo (+ folded bv @ wo).

Fast path (causal mask, zero q/k biases):
  - x is pre-transposed on the host; all loads are plain contiguous DMAs
    issued weights-first so the first matmul starts ~4us in.
  - Q/K projections packed into 3 M-tiles: [h0q|h1q], [h0k|h1k],
    [h2q|h2k]; a partition-swapped copy of the last enables row-paired
    h2 QK^T matmuls.
  - QK^T row-paired (two 64-contraction matmuls in disjoint PE row
    groups run concurrently); diagonal 128-blocks are column-trimmed in
    QK^T, exp, and PV.
  - softmax denominators: ones-column in V (free rowsum), fast DVE
    reciprocal, fp16 PE broadcast, one DVE multiply per head.
  - causal tril masking on GpSimd; PSUM time-shared 4/2/2 banks.
"""

import numpy as np

import bass_rust
import concourse.bass as bass
import concourse.mybir as mybir
import concourse.tile as tile
from concourse.bass_utils import run_bass_kernel_spmd

F16 = mybir.dt.float16
F32 = mybir.dt.float32

B, S, D = 2, 2048, 768
H, DK = 12, 64
HPC = 3            # heads per core
N_CORES = 8
QB = 512           # query block (psum free dim)
NQB = S // QB      # 4
NKT = S // 128     # 16 key tiles
DKT = D // 128     # 6 contraction tiles for projections

ScopedClock = bass_rust.ScopedClock


# ---------------------------------------------------------------------------
# walrus in this build accepts at most ONE sync-wait per instruction; spread
# extra waits onto NOPs placed immediately before the owning instruction.

def _split_drain_and_barrier(self, tick_clock, wait_clock):
    probe = self.nc.sync.nop()
    wait_clock.add_sem_waits(probe.ins, ScopedClock({None: tick_clock.global_clock}))
    si = probe.ins.sync_info
    waits = list(si.on_wait) if si is not None else []
    if len(waits) > 1:
        si.on_wait = waits[:1]
        for w in waits[1:]:
            n = self.nc.sync.nop()
            nsi = n.ins.sync_info
            if nsi is None:
                n.ins.sync_info = bass_rust.SyncInfo(on_wait=[w], on_update=[])
            else:
                nsi.on_wait = [w]
    self.nc.sync.drain()

    self.nc.all_engine_barrier()
    assert self.sems is not None
    popped = self.nc._tile_sem_poison_stack.pop()
    assert popped is self._sem_poison
    self.nc.clear_and_free_semaphores(list(self.sems.allocated().values()))
    self.nc.all_engine_barrier()


tile.TileContext._drain_and_barrier = _split_drain_and_barrier

_nop_ctr = [0]


def split_multi_waits(nc):
    def visit(parent):
        for bb in parent.blocks:
            insts = bb.instructions
            out = []
            changed = False
            for inst in insts:
                si = inst.sync_info
                if si is not None and len(si.on_wait) > 1:
                    waits = list(si.on_wait)
                    for w in waits[:-1]:
                        _nop_ctr[0] += 1
                        nop = mybir.InstNoOp(
                            name=f"wsplit{_nop_ctr[0]}",
                            sync_info=mybir.SyncInfo(on_wait=[w], on_update=[]),
                            bass_nofuse=True,
                            engine=inst.engine,
                        )
                        out.append(nop)
                    si.on_wait = waits[-1:]
                    changed = True
                out.append(inst)
            if changed:
                bb.instructions = out
    for f in nc.m.functions:
        visit(f)


# ---------------------------------------------------------------------------
# fast causal kernel


def build_nc_fast(with_qk_bias: bool = False):
    nc = bass.Bass("TRN2", target_bir_lowering=False, debug=False,
                   num_devices=N_CORES)

    xTd = nc.dram_tensor("xT", (128, DKT, S), F16, kind="ExternalInput").ap()
    wqkd = nc.dram_tensor("wqk", (128, DKT, 3, 128), F16, kind="ExternalInput").ap()
    wvd = nc.dram_tensor("wv3", (128, DKT, HPC * DK), F16, kind="ExternalInput").ap()
    wo2d = nc.dram_tensor("wo2", (128, D), F16, kind="ExternalInput").ap()
    wo3d = nc.dram_tensor("wo3", (128, D), F16, kind="ExternalInput").ap()
    if with_qk_bias:
        bqkd = nc.dram_tensor("bqk3", (128, 3), F32, kind="ExternalInput").ap()
    o16 = nc.dram_tensor("o16", (S, D), F16, kind="ExternalOutput").ap()

    with tile.TileContext(nc) as tc:
        cst = tc.alloc_tile_pool(name="cst", bufs=1)
        psA2 = tc.alloc_tile_pool(name="psA2", bufs=2, space="PSUM")  # 4 banks
        psA = tc.alloc_tile_pool(name="psA", bufs=2, space="PSUM")    # 2 banks
        psC = tc.alloc_tile_pool(name="psC", bufs=2, space="PSUM")    # 2 banks
        c1s_pool = tc.alloc_tile_pool(name="c1s", bufs=2)
        osb_pool = tc.alloc_tile_pool(name="osbp", bufs=3)

        # ---- constants / weights first (small), then x chunks
        wqk_sb = cst.tile([128, DKT, 3, 128], F16, tag="wqk")
        wv_sb = cst.tile([128, DKT, HPC * DK], F16, tag="wv")
        wo2_sb = cst.tile([128, D], F16, tag="wo2")
        wo3_sb = cst.tile([128, D], F16, tag="wo3")
        for m in (1, 0, 2):
            nc.sync.dma_start(wqk_sb[:, :, m, :], wqkd[:, :, m, :])
        nc.sync.dma_start(wv_sb[:], wvd)
        nc.sync.dma_start(wo2_sb[:], wo2d)
        nc.sync.dma_start(wo3_sb[:], wo3d)
        if with_qk_bias:
            bqk_sb = cst.tile([128, 3], F32, tag="bqk")
            nc.sync.dma_start(bqk_sb[:], bqkd)

        xt = cst.tile([128, DKT, S], F16, tag="xt")
        for c in range(DKT):
            eng = nc.scalar if c % 2 == 0 else nc.sync
            eng.dma_start(xt[:, c, :], xTd[:, c, :])

        tril = cst.tile([128, 128], F16, tag="tril")
        nc.gpsimd.memset(tril[:], 1.0)
        # keep (f - p >= 0) i.e. q >= k, else 0
        nc.gpsimd.affine_select(
            out=tril[:], in_=tril[:], compare_op=mybir.AluOpType.is_ge,
            fill=0.0, base=0, pattern=[[1, 128]], channel_multiplier=-1)

        ones16 = cst.tile([128, DK], F16, tag="ones16")
        nc.vector.memset(ones16[:], 1.0)

        V_sb = cst.tile([128, NKT, HPC, DK + 1], F16, tag="V")
        nc.gpsimd.memset(V_sb[:, :, :, DK:DK + 1], 1.0)  # rowsum ones-column

        qT01 = cst.tile([128, S], F16, tag="qT01")
        kT01 = cst.tile([128, S], F16, tag="kT01")
        qkT2a = cst.tile([128, S], F16, tag="qkT2a")
        qkT2b = cst.tile([128, S], F16, tag="qkT2b")

        # double-buffered per i-parity so exp(i+1) overlaps PV(i)
        pts2 = [[cst.tile([128, NKT, QB], F16, tag=f"pt{h}_{par}",
                          name=f"pt{h}_{par}") for h in range(HPC)]
                for par in range(2)]

        c01 = cst.tile([128, S], F16, tag="c01")
        ctx2 = cst.tile([128, S], F16, tag="ctx2")
        rs32 = cst.tile([128, HPC, QB], F32, tag="rs32")
        rs16 = cst.tile([128, HPC, QB], F16, tag="rs16")

        # ---- emission helpers -------------------------------------------
        dsts = (qT01, kT01, qkT2a)

        def proj_chain(m, sb):
            pp = psA.tile([128, QB], F32, tag="A", name=f"pp{m}_{sb}")
            for c in range(DKT):
                nc.tensor.matmul(
                    pp[:], wqk_sb[:, c, m, :],
                    xt[:, c, sb * QB:(sb + 1) * QB],
                    start=(c == 0), stop=(c == DKT - 1))
            dcols = dsts[m][:, sb * QB:(sb + 1) * QB]
            if with_qk_bias:
                nc.vector.tensor_scalar_add(dcols, pp[:], bqk_sb[:, m:m + 1])
            else:
                nc.vector.tensor_copy(dcols, pp[:])

        def v_chain(st):
            pp = psA.tile([128, QB], F32, tag="A", name=f"ppv{st}")
            for c in range(DKT):
                nc.tensor.matmul(
                    pp[:, 0:HPC * DK], xt[:, c, st * 128:(st + 1) * 128],
                    wv_sb[:, c, :], start=(c == 0), stop=(c == DKT - 1))
            nc.vector.tensor_copy(
                V_sb[:, st, :, 0:DK],
                pp[:, 0:HPC * DK].rearrange("p (h d) -> p h d", d=DK))

        def off_of(i, j):
            return 128 * (j - 4 * i) if j >= 4 * i else 0

        def qk01(i, g2):
            q0 = i * QB
            pt0, pt1 = pts2[i % 2][0], pts2[i % 2][1]
            j0, j1 = 2 * g2, 2 * g2 + 1
            off0 = off_of(i, j0)
            sA = psA2.tile([128, 2, QB], F32, tag="A2", name=f"sA{i}_{g2}")
            sB = psA2.tile([128, 2, QB], F32, tag="A2", name=f"sB{i}_{g2}")
            for jj, j in ((0, j0), (1, j1)):
                off = off_of(i, j)
                nc.tensor.matmul(
                    sA[:, jj, off:], kT01[0:DK, j * 128:(j + 1) * 128],
                    qT01[0:DK, q0 + off:q0 + QB], start=True, stop=True)
                nc.tensor.matmul(
                    sB[:, jj, off:], kT01[DK:128, j * 128:(j + 1) * 128],
                    qT01[DK:128, q0 + off:q0 + QB], start=True, stop=True)
            # one exp per [128,2,QB] tile; cols < off1 of plane j1 are
            # garbage-but-never-read (PV/tril only touch cols >= off)
            nc.scalar.activation(pt0[:, j0:j0 + 2, off0:], sA[:, :, off0:],
                                 mybir.ActivationFunctionType.Exp)
            nc.scalar.activation(pt1[:, j0:j0 + 2, off0:], sB[:, :, off0:],
                                 mybir.ActivationFunctionType.Exp)

        def qk2(i, g2):
            q0 = i * QB
            pt2 = pts2[i % 2][2]
            j0, j1 = 2 * g2, 2 * g2 + 1
            off0, off1 = off_of(i, j0), off_of(i, j1)
            sC = psA2.tile([128, 2, QB], F32, tag="A2", name=f"sC{i}_{g2}")
            nc.tensor.matmul(
                sC[:, 0, off0:], qkT2b[0:DK, j0 * 128:(j0 + 1) * 128],
                qkT2a[0:DK, q0 + off0:q0 + QB], start=True, stop=True)
            nc.tensor.matmul(
                sC[:, 1, off1:], qkT2a[DK:128, j1 * 128:(j1 + 1) * 128],
                qkT2b[DK:128, q0 + off1:q0 + QB], start=True, stop=True)
            nc.scalar.activation(pt2[:, j0:j0 + 2, off0:], sC[:, :, off0:],
                                 mybir.ActivationFunctionType.Exp)

        pvs_of = {}

        def mask_pv(i):
            """tril mask (GpSimd) + PV accumulation chains (PE)."""
            kt = 4 * (i + 1)
            pts = pts2[i % 2]
            for h in range(HPC):
                for jj in range(4):
                    j = 4 * i + jj
                    off = 128 * jj
                    nc.gpsimd.tensor_mul(
                        pts[h][:, j, off:off + 128],
                        pts[h][:, j, off:off + 128], tril[:])
            pvs = []
            for h in range(HPC):
                pv = psC.tile([128, QB], F32, tag="C", name=f"pv{i}_{h}")
                pvs.append(pv)
                for j in range(kt):
                    off = off_of(i, j)
                    nc.tensor.matmul(
                        pv[0:DK + 1, off:QB], V_sb[:, j, h, :],
                        pts[h][:, j, off:QB],
                        start=(j == 0), stop=(j == kt - 1),
                        skip_group_check=True)
            pvs_of[i] = pvs

        def recip_bcmul(i):
            """1/r = exp(-ln r) on ACT (same table as the exp stream), PE
            broadcast, SBUF stage, DVE normalize multiply."""
            q0 = i * QB
            pvs = pvs_of[i]
            for h in range(HPC):
                nc.scalar.activation(rs32[DK:DK + 1, h, :],
                                     pvs[h][DK:DK + 1, :],
                                     mybir.ActivationFunctionType.Ln)
                nc.scalar.activation(rs16[DK:DK + 1, h, :],
                                     rs32[DK:DK + 1, h, :],
                                     mybir.ActivationFunctionType.Exp,
                                     scale=-1.0)
            for h in range(HPC):
                pv = pvs[h]
                bc = psA.tile([128, QB], F32, tag="A", name=f"bc{i}_{h}")
                nc.tensor.matmul(bc[0:DK, :], ones16[DK:DK + 1, 0:DK],
                                 rs16[DK:DK + 1, h, :], start=True, stop=True)
                bcs = c1s_pool.tile([DK, QB], F32, tag="bcs", name=f"bcs{i}_{h}")
                nc.vector.tensor_copy(bcs[:], bc[0:DK, :])
                if h == 0:
                    nc.vector.tensor_mul(c01[0:DK, q0:q0 + QB], pv[0:DK, :],
                                         bcs[:])
                    continue
                c1s = c1s_pool.tile([DK, QB], F16, tag=f"c1s{h}",
                                    name=f"c1s{i}_{h}")
                nc.vector.tensor_mul(c1s[:], pv[0:DK, :], bcs[:])
                if h == 1:
                    nc.sync.dma_start(c01[DK:128, q0:q0 + QB], c1s[:])
                else:
                    # h2 ctx: even 128-chunks at partitions 0-63, odd at
                    # 64-127, so the out-proj h2 matmuls pair across chunks
                    ev = ctx2[0:DK, q0:q0 + QB].rearrange(
                        "p (c t k) -> p c t k", t=2, k=128)
                    sv = c1s.rearrange("p (c t k) -> p c t k", t=2, k=128)
                    nc.sync.dma_start(ev[:, :, 0, :], sv[:, :, 0, :])
                    od = ctx2[DK:128, q0:q0 + QB].rearrange(
                        "p (c t k) -> p c t k", t=2, k=128)
                    nc.sync.dma_start(od[:, :, 1, :], sv[:, :, 1, :])

        def outproj_pair(ce):
            """Output projection for chunks (ce, ce+1); the two h2 matmuls
            contract on disjoint partition halves and run concurrently."""
            co = ce + 1
            csl_e = slice(ce * 128, (ce + 1) * 128)
            csl_o = slice(co * 128, (co + 1) * 128)
            osb_e = osb_pool.tile([128, D], F16, tag="osb", name=f"osb{ce}")
            osb_o = osb_pool.tile([128, D], F16, tag="osb", name=f"osb{co}")
            for nb, ncols in ((0, 512), (512, 256)):
                po_e = psA.tile([128, QB], F32, tag="A", name=f"po{ce}_{nb}")
                po_o = psA.tile([128, QB], F32, tag="A", name=f"po{co}_{nb}")
                nc.tensor.matmul(po_e[:, 0:ncols], c01[:, csl_e],
                                 wo2_sb[:, nb:nb + ncols],
                                 start=True, stop=False)
                nc.tensor.matmul(po_o[:, 0:ncols], c01[:, csl_o],
                                 wo2_sb[:, nb:nb + ncols],
                                 start=True, stop=False)
                nc.tensor.matmul(po_e[:, 0:ncols], ctx2[0:DK, csl_e],
                                 wo3_sb[0:DK, nb:nb + ncols],
                                 start=False, stop=True)
                nc.tensor.matmul(po_o[:, 0:ncols], ctx2[DK:128, csl_o],
                                 wo3_sb[DK:128, nb:nb + ncols],
                                 start=False, stop=True)
                nc.vector.tensor_copy(osb_e[:, nb:nb + ncols], po_e[:, 0:ncols])
                nc.vector.tensor_copy(osb_o[:, nb:nb + ncols], po_o[:, 0:ncols])
            nc.sync.dma_start(o16[csl_e, :], osb_e[:])
            nc.sync.dma_start(o16[csl_o, :], osb_o[:])

        # ---- schedule: projections, V, then per-block attention with the
        # previous block's output projection interleaved into the QK stream
        for m in range(3):
            for sb in range(NQB):
                proj_chain(m, sb)
        for st in range(NKT):
            v_chain(st)
        # partition-swapped copy of [h2q|h2k] for paired h2 QK^T
        nc.sync.dma_start(qkT2b[0:DK, :], qkT2a[DK:128, :])
        nc.sync.dma_start(qkT2b[DK:128, :], qkT2a[0:DK, :])

        for i in range(NQB):
            fill = [4 * (i - 1), 4 * (i - 1) + 2] if i > 0 else []
            for g2 in range(2 * (i + 1)):
                qk01(i, g2)
                if fill:
                    outproj_pair(fill.pop(0))
            for g2 in range(2 * (i + 1)):
                qk2(i, g2)
                if fill:
                    outproj_pair(fill.pop(0))
            mask_pv(i)
            recip_bcmul(i)
        outproj_pair(12)
        outproj_pair(14)

        for p in (osb_pool, c1s_pool, psC, psA, psA2, cst):
            p.release()

    split_multi_waits(nc)
    return nc


def _core_inputs_fast(x, wq, bq, wk, bk, wv, bv, wo, with_qk_bias):
    wq8 = (wq * 0.125).astype(np.float32)
    bq8 = (bq * 0.125).astype(np.float32)
    ins = []
    for core in range(N_CORES):
        b, g = divmod(core, 4)
        hs = [HPC * g + k for k in range(HPC)]
        cols = lambda w, h: w[:, h * DK:(h + 1) * DK]
        seg = lambda v, h: v[h * DK:(h + 1) * DK]

        xT = np.ascontiguousarray(x[b].T).reshape(DKT, 128, S)
        xT = np.ascontiguousarray(xT.transpose(1, 0, 2)).astype(np.float16)

        w3 = np.stack([
            np.concatenate([cols(wq8, hs[0]), cols(wq8, hs[1])], axis=1),
            np.concatenate([cols(wk, hs[0]), cols(wk, hs[1])], axis=1),
            np.concatenate([cols(wq8, hs[2]), cols(wk, hs[2])], axis=1),
        ], axis=1)  # [768, 3, 128]
        wqk = np.ascontiguousarray(
            w3.reshape(DKT, 128, 3, 128).transpose(1, 0, 2, 3)).astype(np.float16)

        wv3 = np.concatenate([cols(wv, h) for h in hs], axis=1)
        wv3 = np.ascontiguousarray(
            wv3.reshape(DKT, 128, HPC * DK).transpose(1, 0, 2)).astype(np.float16)

        wo2 = np.concatenate([wo[hs[0] * DK:(hs[0] + 1) * DK],
                              wo[hs[1] * DK:(hs[1] + 1) * DK]]).astype(np.float16)
        wo3h = wo[hs[2] * DK:(hs[2] + 1) * DK]
        wo3 = np.concatenate([wo3h, wo3h]).astype(np.float16)

        m = {"xT": xT, "wqk": wqk, "wv3": wv3, "wo2": wo2, "wo3": wo3}
        if with_qk_bias:
            bqk3 = np.stack([
                np.concatenate([seg(bq8, hs[0]), seg(bq8, hs[1])]),
                np.concatenate([seg(bk, hs[0]), seg(bk, hs[1])]),
                np.concatenate([seg(bq8, hs[2]), seg(bk, hs[2])]),
            ], axis=1).astype(np.float32)
            m["bqk3"] = bqk3
        ins.append(m)
    return ins


# ---------------------------------------------------------------------------
# fallback kernel (arbitrary mask): mask multiply path from the baseline


def build_nc_fallback():
    nc = bass.Bass("TRN2", target_bir_lowering=False, debug=False,
                   num_devices=N_CORES)

    x16 = nc.dram_tensor("x16", (S, D), F16, kind="ExternalInput").ap()
    wqe = nc.dram_tensor("wqe", (D, 256), F16, kind="ExternalInput").ap()
    wke = nc.dram_tensor("wke", (D, 256), F16, kind="ExternalInput").ap()
    wve = nc.dram_tensor("wve", (D, HPC * DK), F16, kind="ExternalInput").ap()
    woe = nc.dram_tensor("woe", (DK, HPC, D), F16, kind="ExternalInput").ap()
    woe2 = nc.dram_tensor("woe2", (128, D), F16, kind="ExternalInput").ap()
    bqk = nc.dram_tensor("bqk", (128, 4), F32, kind="ExternalInput").ap()
    bvp = nc.dram_tensor("bvp", (DK, HPC), F32, kind="ExternalInput").ap()
    mTd = nc.dram_tensor("mT", (S, S), F16, kind="ExternalInput").ap()
    o16 = nc.dram_tensor("o16", (S, D), F16, kind="ExternalOutput").ap()

    with tile.TileContext(nc) as tc:
        cst = tc.alloc_tile_pool(name="cst", bufs=1)
        ps_gen = tc.alloc_tile_pool(name="psg", bufs=2, space="PSUM")
        ps_sc = tc.alloc_tile_pool(name="pss", bufs=2, space="PSUM")
        ps_pv = tc.alloc_tile_pool(name="psv", bufs=2, space="PSUM")
        pt_pool = tc.alloc_tile_pool(name="ptp", bufs=2)
        ctx_pool = tc.alloc_tile_pool(name="ctp", bufs=2)
        rs_pool = tc.alloc_tile_pool(name="rsp", bufs=2)
        out_pool = tc.alloc_tile_pool(name="outp", bufs=3)
        msk_pool = tc.alloc_tile_pool(name="mskp", bufs=2)

        xT = cst.tile([128, DKT, S], F16, tag="xT")
        for c in range(DKT):
            nc.sync.dma_start_transpose(xT[:, c, :], x16[:, c * 128:(c + 1) * 128])

        wq_sb = cst.tile([128, DKT, 256], F16, tag="wq")
        wk_sb = cst.tile([128, DKT, 256], F16, tag="wk")
        wv_sb = cst.tile([128, DKT, HPC * DK], F16, tag="wv")
        wo_sb = cst.tile([DK, HPC, D], F16, tag="wo")
        wo_sb2 = cst.tile([128, D], F16, tag="wo2")
        nc.sync.dma_start(wq_sb[:], wqe.rearrange("(kt p) m -> p kt m", p=128))
        nc.sync.dma_start(wk_sb[:], wke.rearrange("(kt p) m -> p kt m", p=128))
        nc.sync.dma_start(wv_sb[:], wve.rearrange("(kt p) m -> p kt m", p=128))
        nc.sync.dma_start(wo_sb[:], woe)
        nc.sync.dma_start(wo_sb2[:], woe2)
        bqk_sb = cst.tile([128, 4], F32, tag="bqk")
        bv_sb = cst.tile([DK, HPC], F32, tag="bv")
        nc.sync.dma_start(bqk_sb[:], bqk)
        nc.sync.dma_start(bv_sb[:], bvp)

        ones = cst.tile([128, DK], F32, tag="ones")
        nc.vector.memset(ones[:], 1.0)

        V_sb = cst.tile([128, NKT, HPC, DK + 1], F16, tag="V")
        nc.vector.memset(V_sb[:], 1.0)  # pre-sets the rowsum ones-column

        qT0 = cst.tile([128, S], F16, tag="qT0")
        qT1 = cst.tile([128, S], F16, tag="qT1")
        kT0 = cst.tile([128, S], F16, tag="kT0")
        kT1 = cst.tile([128, S], F16, tag="kT1")

        for wsb, dsts, bcol in ((wq_sb, (qT0, qT1), 0), (wk_sb, (kT0, kT1), 2)):
            for t in range(2):
                for sb in range(NQB):
                    ps = ps_gen.tile([128, QB], F32, tag="psg", name=f"psp{t}{sb}")
                    for c in range(DKT):
                        nc.tensor.matmul(
                            ps[:], wsb[:, c, t * 128:(t + 1) * 128],
                            xT[:, c, sb * QB:(sb + 1) * QB],
                            start=(c == 0), stop=(c == DKT - 1))
                    nc.vector.tensor_scalar_add(
                        dsts[t][:, sb * QB:(sb + 1) * QB], ps[:],
                        bqk_sb[:, bcol + t:bcol + t + 1])

        for st in range(NKT):
            ps = ps_gen.tile([128, QB], F32, tag="psg", name=f"psv{st}")
            for c in range(DKT):
                nc.tensor.matmul(
                    ps[:, 0:HPC * DK], xT[:, c, st * 128:(st + 1) * 128],
                    wv_sb[:, c, :], start=(c == 0), stop=(c == DKT - 1))
            nc.vector.tensor_copy(
                V_sb[:, st, :, 0:DK],
                ps[:, 0:HPC * DK].rearrange("p (h d) -> p h d", d=DK))

        for i in range(NQB):
            ctx = ctx_pool.tile([DK, HPC, QB], F16, tag="ctx")
            c01 = ctx_pool.tile([128, QB], F16, tag="c01")
            mtile = msk_pool.tile([128, NKT, QB], F16, tag="mt")
            nc.sync.dma_start(
                mtile[:],
                mTd.rearrange("(kt p) q -> p kt q", p=128)[:, :, i * QB:(i + 1) * QB])
        # (kept identical to the baseline fallback loop below)
            kt = NKT
            qcols = slice(i * QB, (i + 1) * QB)
            pts = [pt_pool.tile([128, NKT, QB], F16, tag=f"pt{h}",
                                name=f"pt{h}") for h in range(HPC)]
            for g2 in range(kt // 2):
                scA = ps_sc.tile([128, 2, QB], F32, tag="sc", name="scA")
                scB = ps_sc.tile([128, 2, QB], F32, tag="sc", name="scB")
                for jj in range(2):
                    j = 2 * g2 + jj
                    nc.tensor.matmul(
                        scA[:, jj, :], kT0[0:DK, j * 128:(j + 1) * 128],
                        qT0[0:DK, qcols], start=True, stop=True,
                        tile_position=(0, 0))
                    nc.tensor.matmul(
                        scB[:, jj, :], kT0[DK:128, j * 128:(j + 1) * 128],
                        qT0[DK:128, qcols], start=True, stop=True,
                        tile_position=(DK, 0))
                nc.scalar.activation(pts[0][:, 2 * g2:2 * g2 + 2, :], scA[:],
                                     mybir.ActivationFunctionType.Exp)
                nc.scalar.activation(pts[1][:, 2 * g2:2 * g2 + 2, :], scB[:],
                                     mybir.ActivationFunctionType.Exp)
            for g2 in range(kt // 2):
                scC = ps_sc.tile([128, 2, QB], F32, tag="sc", name="scC")
                j0, j1 = 2 * g2, 2 * g2 + 1
                nc.tensor.matmul(
                    scC[:, 0, :], kT1[0:DK, j0 * 128:(j0 + 1) * 128],
                    qT1[0:DK, qcols], start=True, stop=True,
                    tile_position=(0, 0))
                nc.tensor.matmul(
                    scC[:, 1, :], kT1[DK:128, j1 * 128:(j1 + 1) * 128],
                    qT1[DK:128, qcols], start=True, stop=True,
                    tile_position=(DK, 0))
                nc.scalar.activation(pts[2][:, 2 * g2:2 * g2 + 2, :], scC[:],
                                     mybir.ActivationFunctionType.Exp)
            for h in range(HPC):
                pt = pts[h]
                for j in range(NKT):
                    nc.vector.tensor_mul(
                        pt[:, j, :], pt[:, j, :], mtile[:, j, :])

                pv = ps_pv.tile([128, QB], F32, tag="pv", name="pv")
                for j in range(kt):
                    nc.tensor.matmul(
                        pv[0:DK + 1, :], V_sb[:, j, h, :],
                        pt[:, j, :],
                        start=(j == 0), stop=(j == kt - 1),
                        skip_group_check=True)

                dst = c01[0:DK, :] if h == 0 else ctx[:, h, :]
                rs = rs_pool.tile([128, QB], F32, tag="rs", name="rs")
                nc.vector.reciprocal(rs[DK:DK + 1, :], pv[DK:DK + 1, :])
                bc = ps_gen.tile([128, QB], F32, tag="psg", name="bc")
                nc.tensor.matmul(bc[0:DK, :], ones[DK:DK + 1, 0:DK],
                                 rs[DK:DK + 1, :], start=True, stop=True,
                                 tile_position=(DK, 0))
                bcs = rs_pool.tile([DK, QB], F32, tag="bcs", name="bcs")
                nc.vector.tensor_copy(bcs[:], bc[0:DK, :])
                nc.vector.tensor_mul(dst, pv[0:DK, :], bcs[:])
                nc.vector.tensor_scalar_add(dst, dst, bv_sb[:, h:h + 1])

            nc.sync.dma_start(c01[DK:128, :], ctx[:, 1, :])
            for cch in range(QB // 128):
                chunk = i * (QB // 128) + cch
                csl = slice(cch * 128, (cch + 1) * 128)
                osb = out_pool.tile([128, D], F16, tag="osb", name="osb")
                for nb, ncols in ((0, 512), (512, 256)):
                    ps = ps_gen.tile([128, QB], F32, tag="psg", name="pso")
                    nc.tensor.matmul(ps[:, 0:ncols], c01[:, csl],
                                     wo_sb2[:, nb:nb + ncols],
                                     start=True, stop=False)
                    nc.tensor.matmul(ps[:, 0:ncols], ctx[:, 2, csl],
                                     wo_sb[:, 2, nb:nb + ncols],
                                     start=False, stop=True)
                    nc.vector.tensor_copy(osb[:, nb:nb + ncols], ps[:, 0:ncols])
                nc.sync.dma_start(o16[chunk * 128:(chunk + 1) * 128, :], osb[:])

        pools = [cst, ps_gen, ps_sc, ps_pv, pt_pool, ctx_pool, rs_pool,
                 out_pool, msk_pool]
        for p in reversed(pools):
            p.release()

    split_multi_waits(nc)
    return nc


def _core_inputs_fallback(x, mask, wq, bq, wk, bk, wv, bv, wo):
    ins = []
    wq8 = (wq * 0.125).astype(np.float32)
    bq8 = (bq * 0.125).astype(np.float32)
    mT = (mask[0, 0].T != 0).astype(np.float16)
    for core in range(N_CORES):
        b, g = divmod(core, 4)
        hs = [HPC * g + k for k in range(HPC)]
        cols = lambda w, h: w[:, h * DK:(h + 1) * DK]

        wqe = np.concatenate(
            [cols(wq8, hs[0]), cols(wq8, hs[1]), cols(wq8, hs[2]), cols(wq8, hs[2])],
            axis=1).astype(np.float16)
        wke = np.concatenate(
            [cols(wk, hs[0]), cols(wk, hs[1]), cols(wk, hs[2]), cols(wk, hs[2])],
            axis=1).astype(np.float16)
        wve = np.concatenate([cols(wv, h) for h in hs], axis=1).astype(np.float16)
        woe = wo.reshape(H, DK, D)[hs].transpose(1, 0, 2).astype(np.float16)
        woe2 = np.concatenate([wo[hs[0] * DK:(hs[0] + 1) * DK],
                               wo[hs[1] * DK:(hs[1] + 1) * DK]]).astype(np.float16)
        seg = lambda v, h: v[h * DK:(h + 1) * DK]
        bqk_pack = np.stack([
            np.concatenate([seg(bq8, hs[0]), seg(bq8, hs[1])]),
            np.concatenate([seg(bq8, hs[2]), seg(bq8, hs[2])]),
            np.concatenate([seg(bk, hs[0]), seg(bk, hs[1])]),
            np.concatenate([seg(bk, hs[2]), seg(bk, hs[2])]),
        ], axis=1).astype(np.float32)
        bvp = np.stack([seg(bv, h) for h in hs], axis=1).astype(np.float32)

        ins.append({
            "x16": x[b].astype(np.float16),
            "wqe": wqe, "wke": wke, "wve": wve, "woe": woe, "woe2": woe2,
            "bqk": bqk_pack, "bvp": bvp, "mT": mT,
        })
    return ins


# ---------------------------------------------------------------------------

_CACHE = {}


def _get_nc(key):
    if key not in _CACHE:
        if key == "fallback":
            _CACHE[key] = build_nc_fallback()
        else:
            _CACHE[key] = build_nc_fast(with_qk_bias=(key == "fast_bias"))
    return _CACHE[key]


def _prep(x, mask, wq, bq, wk, bk, wv, bv, wo):
    """Returns (nc, per-core input maps, mode string)."""
    m2 = np.asarray(mask[0, 0])
    causal = bool(np.array_equal(m2, np.tril(np.ones((S, S), m2.dtype))))
    if causal:
        with_bias = bool(np.any(bq) or np.any(bk))
        key = "fast_bias" if with_bias else "fast"
        nc = _get_nc(key)
        ins = _core_inputs_fast(x, wq, bq, wk, bk, wv, bv, wo, with_bias)
        return nc, ins, key
    nc = _get_nc("fallback")
    ins = _core_inputs_fallback(x, mask, wq, bq, wk, bk, wv, bv, wo)
    return nc, ins, "fallback"


def kernel(x, mask, wq, bq, wk, bk, wv, bv, wo, bo):
    x = np.asarray(x)
    mask = np.asarray(mask)
    wq, bq = np.asarray(wq), np.asarray(bq)
    wk, bk = np.asarray(wk), np.asarray(bk)
    wv, bv = np.asarray(wv), np.asarray(bv)
    wo, bo = np.asarray(wo), np.asarray(bo)
    nc, ins, mode = _prep(x, mask, wq, bq, wk, bk, wv, bv, wo)
    res = run_bass_kernel_spmd(nc, ins, core_ids=list(range(N_CORES)))
    out = np.zeros((B, S, D), np.float32)
    for core in range(N_CORES):
        b = core // 4
        out[b] += res.results[core]["o16"].astype(np.float32)
    out += np.asarray(bo, np.float32)
    if mode != "fallback":
        # bv was folded out of the device kernel: ctx/r + 1*bv^T through wo
        out += bv.astype(np.float32) @ wo.astype(np.float32)
    return out
